# revision 6
# baseline (speedup 1.0000x reference)
"""CrossAttention kernel for 8 trn2 NeuronCores.

Reference:
  q = x @ Wq          [n, vq, h]
  k = y @ Wk          [n, vk, h]
  v = y @ Wv          [n, vk, c]
  out = softmax(q k^T / sqrt(h)) @ v        [n, vq, c]
with N=4, VQ=VK=4096, C=128, H=64, fp32.

Sharding: 8 cores = 4 batches x 2 query halves. Each core gets
x-shard [2048, 128] and the full y[n] [4096, 128], computes out-shard
[2048, 128].

Per-core dataflow (matmul operands bf16; fp32 accumulation in PSUM).
The PE front-end streams ~1 moving column/cycle regardless of quadrant
packing, so phase-2 cost = matmul count per chunk. v3 cuts it 8 -> 5:
  - HAM warmup: dummy transposes while DMA loads x/y (PE clock 1.2->2.4)
  - transpose x,y tiles on PE (fp32) -> bf16 SBUF (ScalarE copy-cast)
  - qT [h, vq]; kT [h, vk] (rows 64:128 duplicated); v [vk, c]
  - flash loop over vq tiles of 512, chunks of 2 vk tiles:
      sc pair (2 MM, quadrant-packed)
      attnT = exp(scale * sc)  (ScalarE, PSUM -> SBUF bf16)
      pv += v_tile^T attnT (2 MM, fp32 PSUM accum over 32 vk tiles)
      fold = attnT_i + attnT_{i+1}  (DVE bf16, 2x/4x mode)
      sm += ones^T fold (1 MM, fp32 PSUM accum)
  - per-tile finalize inside the loop: outT bf16, sums row-transpose,
    reciprocal, out transpose (bf16, 1 cyc/row), scale, DMA out
"""

import sys

sys.path.insert(0, "/opt/trn_rl_repo")

from contextlib import ExitStack

import numpy as np

import concourse.bass as bass
import concourse.tile as tile
from concourse import mybir
from concourse.bass_utils import run_bass_kernel_spmd
from concourse.masks import make_identity

F32 = mybir.dt.float32
BF16 = mybir.dt.bfloat16
P = 128

N, VQ, VK, C, H = 4, 4096, 4096, 128, 64
VQ_PER = VQ // 2          # 2048 queries per core
SCALE = float(H) ** -0.5

# main-loop tiling
VQ_T = 512                # vq tile (psum free dim)
N_VQ_T = VQ_PER // VQ_T   # 4
N_VK_T = VK // P          # 32 vk tiles of 128
CHUNK = 2                 # vk tiles per exp chunk (row-packed pair)
N_WARM = 20               # HAM warmup transposes (~4us busy at cold clock)


def _split_multi_waits(nc):
    """walrus in this env supports one sync-wait per instruction; hoist
    extras onto same-engine NoOps inserted just before."""
    for fn in nc.m.functions:
        for bb in fn.blocks:
            out = []
            for inst in bb.instructions:
                si = inst.sync_info
                waits = list(si.on_wait) if si and si.on_wait else []
                if len(waits) > 1:
                    for w in waits[:-1]:
                        out.append(mybir.InstNoOp(
                            name=nc.get_next_instruction_name(),
                            engine=inst.engine,
                            ins=[], outs=[],
                            sync_info=mybir.SyncInfo(on_wait=[w], on_update=[]),
                        ))
                    inst.sync_info = mybir.SyncInfo(
                        on_wait=[waits[-1]],
                        on_update=list(si.on_update) if si.on_update else [],
                    )
                out.append(inst)
            bb.instructions = out


def _build():
    nc = bass.Bass()
    x_d = nc.declare_dram_parameter("x", [VQ_PER, C], F32, isOutput=False)
    y_d = nc.declare_dram_parameter("y", [VK, C], F32, isOutput=False)
    wq_d = nc.declare_dram_parameter("Wq", [C, H], F32, isOutput=False)
    wk_d = nc.declare_dram_parameter("Wk", [C, H], F32, isOutput=False)
    wv_d = nc.declare_dram_parameter("Wv", [C, C], F32, isOutput=False)
    o_d = nc.declare_dram_parameter("o", [VQ_PER, C], F32, isOutput=True)

    with tile.TileContext(nc) as tc, ExitStack() as ctx:
        const = ctx.enter_context(tc.tile_pool(name="const", bufs=1))
        persist = ctx.enter_context(tc.tile_pool(name="persist", bufs=1))

        # ---- constants ----
        ident = const.tile([P, P], F32)
        make_identity(nc, ident[:])
        w_stage = const.tile([P, 2 * H + C], F32)
        nc.sync.dma_start(w_stage[:, 0:H], wq_d[:])
        nc.sync.dma_start(w_stage[:, H:2 * H], wk_d[:])
        nc.sync.dma_start(w_stage[:, 2 * H:], wv_d[:])
        w_r = const.tile([P, 4 * H + C], BF16)
        nc.vector.tensor_copy(w_r[:, 0:H], w_stage[:, 0:H])
        nc.vector.tensor_copy(w_r[:, H:2 * H], w_stage[:, 0:H])
        nc.vector.tensor_copy(w_r[:, 2 * H:3 * H], w_stage[:, H:2 * H])
        nc.vector.tensor_copy(w_r[:, 3 * H:4 * H], w_stage[:, H:2 * H])
        nc.vector.tensor_copy(w_r[:, 4 * H:], w_stage[:, 2 * H:])
        wqq_r = w_r[:, 0:2 * H]      # [Wq | Wq] -> duplicated qT rows
        wkk_r = w_r[:, 2 * H:4 * H]  # [Wk | Wk] -> duplicated kT rows
        wv_r = w_r[:, 4 * H:]
        ones_f = const.tile([P, 1], F32)
        nc.vector.memset(ones_f[:], 1.0)
        ones_r = const.tile([P, 1], BF16)
        nc.vector.tensor_copy(ones_r[:], ones_f[:])

        # ---- persistent tensors ----
        qT = persist.tile([P, VQ_PER], BF16)          # [128, 2048] rows 64:128 dup
        kT = persist.tile([P, VK], BF16)              # [128, 4096] rows 64:128 dup
        v_sb = persist.tile([P, N_VK_T * C], BF16)    # [128, 32*128] vk-major
        attnT = persist.tile([P, N_VK_T * VQ_T], BF16)  # [128, 32*512] per vq tile
        fold_sb = persist.tile([P, (N_VK_T // 2) * VQ_T], BF16)  # [128, 16*512]
        outT = persist.tile([P, VQ_PER], F32)         # [c, 2048]
        out_sb = persist.tile([P, (VQ_PER // P) * C], F32)  # [128, 16*128]
        srow = persist.tile([1, VQ_PER], F32)         # softmax sums, vq-flat
        rsum_all = persist.tile([P, VQ_PER // P], F32)  # [128, 16] recips

        # ---- phase 1: HAM warmup + load + transpose + project ----
        with ExitStack() as pctx:
            warm_ps = pctx.enter_context(
                tc.tile_pool(name="warm_ps", bufs=1, space="PSUM"))
            ld = pctx.enter_context(tc.tile_pool(name="ld", bufs=6))
            tp_ps = pctx.enter_context(
                tc.tile_pool(name="tp_ps", bufs=2, space="PSUM"))
            pj_ps = pctx.enter_context(
                tc.tile_pool(name="pj_ps", bufs=2, space="PSUM"))
            v_ps = pctx.enter_context(
                tc.tile_pool(name="v_ps", bufs=2, space="PSUM"))
            xyT = pctx.enter_context(tc.tile_pool(name="xyT", bufs=3))

            # dummy transposes: keep PE busy ~3.5us so the HAM clock gate
            # opens (1.2 -> 2.4 GHz) before the real work arrives
            warm = warm_ps.tile([P, P], F32)
            for _ in range(N_WARM):
                nc.tensor.transpose(warm[:], ident[:], ident[:])

            def load_transpose(src_ap, n_chunks, proj):
                for ch in range(n_chunks):
                    raw = ld.tile([P, 4, P], F32, tag="raw")
                    nc.sync.dma_start(
                        raw[:],
                        src_ap[ch * 512:(ch + 1) * 512, :]
                        .rearrange("(t p) c -> p t c", p=P),
                    )
                    t_ps = tp_ps.tile([P, 512], F32, tag="tp")
                    for b in range(4):
                        nc.tensor.transpose(
                            t_ps[:, b * P:(b + 1) * P], raw[:, b, :], ident[:])
                    t_sb = xyT.tile([P, 512], BF16, tag="t_sb")
                    # ScalarE copy-cast PSUM f32 -> SBUF bf16 (keeps DVE free)
                    nc.scalar.activation(
                        t_sb[:], t_ps[:], mybir.ActivationFunctionType.Copy)
                    proj(ch, t_sb)

            def proj_x(ch, xT_sb):
                q_ps = pj_ps.tile([P, 512], F32, tag="qk")
                nc.tensor.matmul(q_ps[:], wqq_r[:], xT_sb[:], start=True, stop=True)
                nc.vector.tensor_copy(qT[:, ch * 512:(ch + 1) * 512], q_ps[:])

            def proj_y(ch, yT_sb):
                k_ps = pj_ps.tile([P, 512], F32, tag="qk")
                nc.tensor.matmul(k_ps[:], wkk_r[:], yT_sb[:], start=True, stop=True)
                nc.vector.tensor_copy(kT[:, ch * 512:(ch + 1) * 512], k_ps[:])
                vp = v_ps.tile([P, 512], F32, tag="vp")
                for b in range(4):
                    nc.tensor.matmul(
                        vp[:, b * P:(b + 1) * P],
                        yT_sb[:, b * P:(b + 1) * P], wv_r[:],
                        start=True, stop=True)
                nc.vector.tensor_copy(
                    v_sb[:, ch * 512:(ch + 1) * 512], vp[:])

            load_transpose(x_d, VQ_PER // 512, proj_x)
            load_transpose(y_d, VK // 512, proj_y)

        # ---- phase 2: flash loop over vq tiles, finalize folded in ----
        with ExitStack() as mctx:
            sc_ps = mctx.enter_context(
                tc.tile_pool(name="sc_ps", bufs=2, space="PSUM"))
            pv_ps = mctx.enter_context(
                tc.tile_pool(name="pv_ps", bufs=2, space="PSUM"))
            sm_ps = mctx.enter_context(
                tc.tile_pool(name="sm_ps", bufs=1, space="PSUM"))
            fin_ps = mctx.enter_context(
                tc.tile_pool(name="fin_ps", bufs=1, space="PSUM"))
            fin = mctx.enter_context(tc.tile_pool(name="fin", bufs=2))

            pv_tiles = [None] * N_VQ_T
            starts = list(range(0, N_VK_T, CHUNK))
            n_st = len(starts)
            work = [(j, s) for j in range(N_VQ_T) for s in starts]

            def emit_scores_exp(j, s):
                sc = sc_ps.tile([P, CHUNK * VQ_T], F32, tag="sc")
                nc.tensor.matmul(
                    sc[:, 0:VQ_T],
                    kT[0:64, s * P:(s + 1) * P],
                    qT[0:64, j * VQ_T:(j + 1) * VQ_T],
                    start=True, stop=True)
                nc.tensor.matmul(
                    sc[:, VQ_T:2 * VQ_T],
                    kT[64:128, (s + 1) * P:(s + 2) * P],
                    qT[64:128, j * VQ_T:(j + 1) * VQ_T],
                    start=True, stop=True, tile_position=(64, 0))
                nc.scalar.activation(
                    attnT[:, s * VQ_T:(s + 2) * VQ_T],
                    sc[:],
                    mybir.ActivationFunctionType.Exp, scale=SCALE)

            def emit_pv_sm(j, s):
                if s == 0:
                    pv = pv_ps.tile([P, VQ_T], F32, tag="pv", name=f"pv{j}")
                    sm = sm_ps.tile([1, VQ_T], F32, tag="sm", name=f"sm{j}")
                    pv_tiles[j] = (pv, sm)
                pv, sm = pv_tiles[j]
                for ii in range(CHUNK):
                    i = s + ii
                    a_sl = attnT[:, i * VQ_T:(i + 1) * VQ_T]
                    nc.tensor.matmul(
                        pv[:], v_sb[:, i * C:(i + 1) * C], a_sl,
                        start=(i == 0), stop=(i == N_VK_T - 1))
                # bf16 pair-fold on DVE (2-byte SBUF operands: fast mode),
                # then a single ones-matmul accumulates the softmax sums
                f_sl = fold_sb[:, (s // 2) * VQ_T:(s // 2 + 1) * VQ_T]
                nc.vector.tensor_tensor(
                    out=f_sl, in0=attnT[:, s * VQ_T:(s + 1) * VQ_T],
                    in1=attnT[:, (s + 1) * VQ_T:(s + 2) * VQ_T],
                    op=mybir.AluOpType.add)
                nc.tensor.matmul(
                    sm[:], ones_r[:], f_sl,
                    start=(s == 0), stop=(s == starts[-1]))

            def emit_finalize(j):
                pv, sm = pv_tiles[j]
                oT = outT[:, j * VQ_T:(j + 1) * VQ_T]
                nc.vector.tensor_copy(oT, pv[:])            # PSUM f32 -> bf16
                sr = srow[:, j * VQ_T:(j + 1) * VQ_T]
                nc.vector.tensor_copy(sr, sm[:])
                n_b = VQ_T // P                              # 4 blocks per tile
                sT_ps = fin_ps.tile([P, VQ_T], F32, tag="fin",
                                    name=f"fin{j}")
                # sums row-transposes use cols 0:n_b first; the out
                # transposes below overwrite them after rst is copied out
                for b in range(n_b):
                    nc.tensor.transpose(
                        sT_ps[:, b:b + 1],
                        sr[0:1, b * P:(b + 1) * P], ones_f[0:1, 0:1])
                rs = rsum_all[:, j * n_b:(j + 1) * n_b]
                rst = fin.tile([P, n_b], F32, tag="rst")
                nc.vector.tensor_copy(rst[:], sT_ps[:, 0:n_b])
                nc.vector.reciprocal(rs, rst[:])
                for b in range(n_b):
                    t = j * n_b + b
                    nc.tensor.transpose(
                        sT_ps[:, b * P:(b + 1) * P],
                        oT[:, b * P:(b + 1) * P], ident[:])
                    nc.vector.tensor_scalar(
                        out=out_sb[:, t * P:(t + 1) * P],
                        in0=sT_ps[:, b * P:(b + 1) * P],
                        scalar1=rs[:, b:b + 1], scalar2=None,
                        op0=mybir.AluOpType.mult)
                nc.sync.dma_start(
                    o_d[j * VQ_T:(j + 1) * VQ_T, :]
                    .rearrange("(t p) c -> p t c", p=P),
                    out_sb[:, j * n_b * C:(j + 1) * n_b * C]
                    .rearrange("p (t c) -> p t c", c=C),
                )

            # software pipeline: sc(n+1) ahead of pv/sm(n); finalize(j)
            # one chunk after tile j's last pv/sm
            fin_due = None
            for n, (j, s) in enumerate(work):
                emit_scores_exp(j, s)
                if n > 0:
                    emit_pv_sm(*work[n - 1])
                    if fin_due is not None:
                        emit_finalize(fin_due)
                        fin_due = None
                    if work[n - 1][1] == starts[-1]:
                        fin_due = work[n - 1][0]
            emit_pv_sm(*work[-1])
            if fin_due is not None:
                emit_finalize(fin_due)
            emit_finalize(work[-1][0])

    _split_multi_waits(nc)
    return nc


_NC = None


def _get_nc():
    global _NC
    if _NC is None:
        _NC = _build()
    return _NC


def kernel(x, y, Wq, Wk, Wv):
    x = np.ascontiguousarray(x, dtype=np.float32)
    y = np.ascontiguousarray(y, dtype=np.float32)
    Wq = np.ascontiguousarray(Wq, dtype=np.float32)
    Wk = np.ascontiguousarray(Wk, dtype=np.float32)
    Wv = np.ascontiguousarray(Wv, dtype=np.float32)

    nc = _get_nc()
    core_ids = list(range(8))
    in_maps = []
    for core in core_ids:
        n, half = core // 2, core % 2
        in_maps.append({
            "x": x[n, half * VQ_PER:(half + 1) * VQ_PER, :],
            "y": y[n],
            "Wq": Wq, "Wk": Wk, "Wv": Wv,
        })
    res = run_bass_kernel_spmd(nc, in_maps, core_ids)
    out = np.empty((N, VQ, C), dtype=np.float32)
    for core in core_ids:
        n, half = core // 2, core % 2
        out[n, half * VQ_PER:(half + 1) * VQ_PER, :] = res.results[core]["o"]
    return out
